# revision 1
# baseline (speedup 1.0000x reference)
"""CrossNetMix (DCN-V2 mixture-of-low-rank-experts) Trainium2 kernel.

Strategy: data-parallel over batch across 8 cores (2048 rows each), with all
tensors kept feature-major on chip ([d, b] layout) so the contraction dim of
every matmul lands on SBUF partitions and no on-chip transposes are needed.
The host pre-transposes each core's x slice and pre-reshapes the weights.

Per layer (fused):
  g = xi @ Wg.T                  -> gating matmuls, M=8
  w = softmax(g)                 -> exp on ACT; partition-sum / broadcast via
                                    tiny ones/selector matmuls on the PE
  h1 = tanh(xi @ Vflat)          -> [er=512, b] feature-major
  h2 = tanh(blockdiag_C @ h1)    -> per-expert C folded into 128x128
                                    block-diagonal pairs (4 matmuls)
  y  = h2 * w_broadcast
  mixed = Uflat.T-style matmul   -> [d, b]
  xi = x0 * (mixed + b) + xi     -> fused combine from PSUM

Matmuls run as float32r (fp32 bits, single-pass PE mode) with fp32 PSUM
accumulation.
"""

import os
import sys

import numpy as np

if "/opt/trn_rl_repo" not in sys.path:
    sys.path.insert(0, "/opt/trn_rl_repo")

import concourse.bass as bass
import concourse.bacc as bacc
import concourse.mybir as mybir
from concourse.tile import TileContext
from concourse.bass_utils import run_bass_kernel_spmd

AF = mybir.ActivationFunctionType
OP = mybir.AluOpType
F32 = mybir.dt.float32

N_CROSS = 3
E = 8            # experts
D = 1024         # feature dim
R = 64           # low rank
B = 16384        # full batch
NCORES = 8
BC = B // NCORES  # rows per core
CHUNK = 512       # batch tile (matmul free dim)
NCHUNK = BC // CHUNK
P = 128
KC = D // P       # d-chunks
ER = E * R        # 512
MC = ER // P      # (e,r)-chunks

MMD = (
    mybir.dt.float32
    if os.environ.get("KMM_DT", "f32r") == "f32"
    else mybir.dt.float32r
)


def _build():
    nc = bacc.Bacc(None)
    xT = nc.declare_dram_parameter("xT", [D, BC], MMD, isOutput=False)
    Vl = nc.declare_dram_parameter("Vl", [N_CROSS, D, ER], MMD, isOutput=False)
    Cb = nc.declare_dram_parameter("Cb", [N_CROSS, MC, P, P], MMD, isOutput=False)
    Ul = nc.declare_dram_parameter("Ul", [N_CROSS, ER, D], MMD, isOutput=False)
    WgT = nc.declare_dram_parameter("WgT", [D, E], MMD, isOutput=False)
    bT = nc.declare_dram_parameter("bT", [N_CROSS, P, KC], F32, isOutput=False)
    sel = nc.declare_dram_parameter("sel", [E, MC + 1, P], MMD, isOutput=False)
    outT = nc.declare_dram_parameter("outT", [D, BC], MMD, isOutput=True)

    with TileContext(nc) as tc:
        with (
            tc.sbuf_pool(name="wpool", bufs=1) as wpool,
            tc.sbuf_pool(name="xpool", bufs=2) as xpool,
            tc.sbuf_pool(name="xipool", bufs=1) as xipool,
            tc.sbuf_pool(name="h1pool", bufs=4) as h1pool,
            tc.sbuf_pool(name="h2pool", bufs=2) as h2pool,
            tc.sbuf_pool(name="ypool", bufs=5) as ypool,
            tc.sbuf_pool(name="tpool", bufs=2) as tpool,
            tc.sbuf_pool(name="spool", bufs=2) as spool,
            tc.psum_pool(name="psmm", bufs=3) as psmm,
            tc.psum_pool(name="psu", bufs=2) as psu,
            tc.psum_pool(name="pswb", bufs=2) as pswb,
            tc.psum_pool(name="psg", bufs=1) as psg,
        ):
            # resident weights, feature-major lhsT layouts. Small tensors
            # first, then layer-0 weights, then the rest — split into
            # per-layer / per-column-chunk DMAs so the first matmuls only
            # wait on the bytes they actually read.
            xTr = xT.rearrange("(kc p) b -> p kc b", p=P)
            outr = outT.rearrange("(kc p) b -> p kc b", p=P)
            Vlr = Vl.rearrange("i (kc p) m -> p i kc m", p=P)
            Ulr = Ul.rearrange("i (mc p) d -> p i mc d", p=P)
            Cbr = Cb.rearrange("i m p s -> p i m s")

            def load_x0(c, parts=2):
                t = xpool.tile([P, KC, CHUNK], MMD, tag="x0", name=f"x0_{c}")
                cbs = slice(c * CHUNK, (c + 1) * CHUNK)
                step = KC // parts
                for q in range(parts):
                    sl = slice(q * step, (q + 1) * step)
                    nc.sync.dma_start(t[:, sl], xTr[:, sl, cbs])
                return t

            x0_tiles = {0: load_x0(0, parts=4)}
            wg_sb = wpool.tile([P, KC, E], MMD)
            nc.sync.dma_start(wg_sb, WgT.rearrange("(kc p) e -> p kc e", p=P))
            sel_sb = wpool.tile([E, MC + 1, P], MMD)
            nc.sync.dma_start(sel_sb, sel[:])
            b_sb = wpool.tile([P, N_CROSS, KC], F32)
            nc.sync.dma_start(b_sb, bT.rearrange("i p kc -> p i kc"))

            v_sb = wpool.tile([P, N_CROSS, KC, ER], MMD)
            u_sb = wpool.tile([P, N_CROSS, MC, D], MMD)
            c_sb = wpool.tile([P, N_CROSS, MC, P], MMD)

            for i in range(N_CROSS):
                for mc in range(MC):
                    nc.sync.dma_start(
                        v_sb[:, i, :, mc * P : (mc + 1) * P],
                        Vlr[:, i, :, mc * P : (mc + 1) * P],
                    )
                nc.sync.dma_start(c_sb[:, i], Cbr[:, i])
                nc.sync.dma_start(u_sb[:, i], Ulr[:, i])

            for c in range(NCHUNK):
                bs = slice(c * CHUNK, (c + 1) * CHUNK)
                x0 = x0_tiles.pop(c)
                if c + 1 < NCHUNK:
                    x0_tiles[c + 1] = load_x0(c + 1)
                xi = xipool.tile([P, KC, CHUNK], MMD, tag="xi")
                for i in range(N_CROSS):
                    src = x0 if i == 0 else xi
                    # ---- gating: g[e, b], then w = softmax over e ----
                    gps = psg.tile([E, CHUNK], F32, tag="g")
                    for kc in range(KC):
                        nc.tensor.matmul(
                            gps,
                            wg_sb[:, kc, :],
                            src[:, kc, :],
                            start=(kc == 0),
                            stop=(kc == KC - 1),
                        )
                    expg = spool.tile([E, CHUNK], MMD, tag="expg")
                    nc.scalar.activation(expg, gps, AF.Exp)
                    sums = psg.tile([1, CHUNK], F32, tag="g")
                    nc.tensor.matmul(
                        sums, sel_sb[:, MC, 0:1], expg, start=True, stop=True
                    )
                    rfast = spool.tile([1, CHUNK], F32, tag="rfast")
                    nc.vector.reciprocal_approx_fast(rfast, sums)
                    rrow = spool.tile([1, CHUNK], MMD, tag="rrow")
                    nc.vector.tensor_copy(rrow, rfast)
                    wps = psg.tile([E, CHUNK], F32, tag="g")
                    nc.tensor.matmul(
                        wps, sel_sb[0:1, MC, 0:E], rrow, start=True, stop=True
                    )
                    wsb = spool.tile([E, CHUNK], MMD, tag="wsb")
                    nc.vector.tensor_tensor(wsb, expg, wps, OP.mult)
                    # ---- V stage: h1 = tanh(Vflat.T @ xi) ----
                    h1s = []
                    for mc in range(MC):
                        vps = psmm.tile([P, CHUNK], F32, tag="mm")
                        for kc in range(KC):
                            nc.tensor.matmul(
                                vps,
                                v_sb[:, i, kc, mc * P : (mc + 1) * P],
                                src[:, kc, :],
                                start=(kc == 0),
                                stop=(kc == KC - 1),
                            )
                        h1 = h1pool.tile([P, CHUNK], MMD, tag="h1")
                        nc.scalar.activation(h1, vps, AF.Tanh)
                        h1s.append(h1)
                    # ---- C stage (block-diag expert pairs) + gate scale ----
                    ys = []
                    for mc in range(MC):
                        cps = psmm.tile([P, CHUNK], F32, tag="mm")
                        nc.tensor.matmul(
                            cps, c_sb[:, i, mc, :], h1s[mc], start=True, stop=True
                        )
                        wbp = pswb.tile([P, CHUNK], F32, tag="wb")
                        nc.tensor.matmul(
                            wbp, sel_sb[:, mc, :], wsb, start=True, stop=True
                        )
                        h2 = h2pool.tile([P, CHUNK], F32, tag="h2")
                        nc.scalar.activation(h2, cps, AF.Tanh)
                        y = ypool.tile([P, CHUNK], MMD, tag="y")
                        nc.vector.tensor_tensor(y, h2, wbp, OP.mult)
                        ys.append(y)
                    # ---- U stage + fused combine ----
                    for dc in range(KC):
                        ups = psu.tile([P, CHUNK], F32, tag="u")
                        for mc in range(MC):
                            nc.tensor.matmul(
                                ups,
                                u_sb[:, i, mc, dc * P : (dc + 1) * P],
                                ys[mc],
                                start=(mc == 0),
                                stop=(mc == MC - 1),
                            )
                        tmp = tpool.tile([P, CHUNK], F32, tag="tmp")
                        nc.vector.scalar_tensor_tensor(
                            tmp,
                            ups,
                            b_sb[:, i, dc : dc + 1],
                            x0[:, dc, :],
                            OP.add,
                            OP.mult,
                        )
                        eng = nc.gpsimd if dc % 2 == 0 else nc.vector
                        eng.tensor_tensor(
                            xi[:, dc, :], tmp, src[:, dc, :], OP.add
                        )
                        if i == N_CROSS - 1:
                            nc.scalar.dma_start(outr[:, dc, bs], xi[:, dc, :])
    nc.compile()
    return nc


_CTX = {}


def _get_nc():
    if "nc" not in _CTX:
        _CTX["nc"] = _build()
    return _CTX["nc"]


def _prep_weights(U, V, C, Wg, b):
    f = np.float32
    U = np.asarray(U, dtype=f)
    V = np.asarray(V, dtype=f)
    C = np.asarray(C, dtype=f)
    Wg = np.asarray(Wg, dtype=f)
    b = np.asarray(b, dtype=f)
    # Vl[i, d, e*R+r] = V[i, e, d, r]
    Vl = np.ascontiguousarray(V.transpose(0, 2, 1, 3).reshape(N_CROSS, D, ER))
    # Ul[i, e*R+r, d] = U[i, e, d, r]
    Ul = np.ascontiguousarray(U.transpose(0, 1, 3, 2).reshape(N_CROSS, ER, D))
    # block-diagonal expert pairs for the C stage
    Cb = np.zeros((N_CROSS, MC, P, P), dtype=f)
    for i in range(N_CROSS):
        for m in range(MC):
            Cb[i, m, :R, :R] = C[i, 2 * m]
            Cb[i, m, R:, R:] = C[i, 2 * m + 1]
    WgT = np.ascontiguousarray(Wg.T)
    # bT[i, p, kc] = b[i, kc*P + p]
    bTa = np.ascontiguousarray(b.reshape(N_CROSS, KC, P).transpose(0, 2, 1))
    # selector planes for broadcasting gate weights over ranks + a ones plane
    sel = np.zeros((E, MC + 1, P), dtype=f)
    for m in range(MC):
        for j in range(P):
            sel[2 * m + j // R, m, j] = 1.0
    sel[:, MC, :] = 1.0
    return dict(Vl=Vl, Ul=Ul, Cb=Cb, WgT=WgT, bT=bTa, sel=sel)


def kernel(x, U, V, C, Wg, b, _trace=False):
    nc = _get_nc()
    w = _prep_weights(U, V, C, Wg, b)
    xs = np.asarray(x, dtype=np.float32).reshape(NCORES, BC, D)
    in_maps = []
    for ci in range(NCORES):
        m = {"xT": np.ascontiguousarray(xs[ci].T)}
        m.update(w)
        in_maps.append(m)
    res = run_bass_kernel_spmd(nc, in_maps, list(range(NCORES)), trace=_trace)
    kernel.last_result = res
    out = np.concatenate(
        [np.asarray(res.results[ci]["outT"]).T for ci in range(NCORES)], axis=0
    )
    return np.ascontiguousarray(out, dtype=np.float32)



# revision 9
# speedup vs baseline: 1.2940x; 1.2940x over previous
"""CrossNetMix (DCN-V2 mixture-of-low-rank-experts) Trainium2 kernel.

Strategy: data-parallel over batch across 8 cores (2048 rows each), with all
tensors kept feature-major on chip ([d, b] layout) so the contraction dim of
every matmul lands on SBUF partitions and no on-chip transposes are needed.
The host pre-transposes each core's x slice and pre-reshapes the weights.

Per layer (fused):
  g = xi @ Wg.T                  -> gating matmuls, M=8
  w = softmax(g)                 -> exp on ACT; partition-sum / broadcast via
                                    tiny ones/selector matmuls on the PE
  h1 = tanh(xi @ Vflat)          -> [er=512, b] feature-major
  h2 = tanh(blockdiag_C @ h1)    -> per-expert C folded into 128x128
                                    block-diagonal pairs (4 matmuls)
  y  = h2 * w_broadcast
  mixed = Uflat.T-style matmul   -> [d, b]
  xi = x0 * (mixed + b) + xi     -> fused combine from PSUM

Matmuls run as float32r (fp32 bits, single-pass PE mode) with fp32 PSUM
accumulation.
"""

import os
import sys

import ml_dtypes
import numpy as np

if "/opt/trn_rl_repo" not in sys.path:
    sys.path.insert(0, "/opt/trn_rl_repo")

import concourse.bass as bass
import concourse.bacc as bacc
import concourse.mybir as mybir
from concourse.tile import TileContext
from concourse.bass_utils import run_bass_kernel_spmd

AF = mybir.ActivationFunctionType
OP = mybir.AluOpType
F32 = mybir.dt.float32

N_CROSS = 3
E = 8            # experts
D = 1024         # feature dim
R = 64           # low rank
B = 16384        # full batch
NCORES = 8
BC = B // NCORES  # rows per core
CHUNK = 512       # batch tile (matmul free dim)
NCHUNK = BC // CHUNK
P = 128
KC = D // P       # d-chunks
ER = E * R        # 512
MC = ER // P      # (e,r)-chunks

MMD = (
    mybir.dt.float32
    if os.environ.get("KMM_DT", "f32r") == "f32"
    else mybir.dt.float32r
)
# stationary (lhsT) operands in bf16: LDWEIGHTS at 1 cyc/row (~53 ns) hides
# under the 512-row matmul instead of serializing ~55 ns/mm like fp32r's.
WDT = mybir.dt.bfloat16
BF16 = ml_dtypes.bfloat16


def _build():
    nc = bacc.Bacc(None)
    xT = nc.declare_dram_parameter("xT", [D, BC], WDT, isOutput=False)
    Vl = nc.declare_dram_parameter("Vl", [N_CROSS, D, ER], WDT, isOutput=False)
    Cb = nc.declare_dram_parameter("Cb", [N_CROSS, MC, P, P], WDT, isOutput=False)
    Ul = nc.declare_dram_parameter("Ul", [N_CROSS, ER, D], WDT, isOutput=False)
    WgT = nc.declare_dram_parameter("WgT", [D, E], WDT, isOutput=False)
    bT = nc.declare_dram_parameter("bT", [N_CROSS, P, KC], F32, isOutput=False)
    sel = nc.declare_dram_parameter("sel", [E, MC + 1, P], WDT, isOutput=False)
    outT = nc.declare_dram_parameter("outT", [D, BC], MMD, isOutput=True)

    with TileContext(nc) as tc:
        with (
            tc.sbuf_pool(name="wpool", bufs=1) as wpool,
            tc.sbuf_pool(name="xpool", bufs=2) as xpool,
            tc.sbuf_pool(name="xipool", bufs=2) as xipool,
            tc.sbuf_pool(name="h1pool", bufs=4) as h1pool,
            tc.sbuf_pool(name="h2pool", bufs=2) as h2pool,
            tc.sbuf_pool(name="ypool", bufs=5) as ypool,
            tc.sbuf_pool(name="tpool", bufs=2) as tpool,
            tc.sbuf_pool(name="spool", bufs=2) as spool,
            tc.psum_pool(name="psmm", bufs=3) as psmm,
            tc.psum_pool(name="psu", bufs=2) as psu,
            tc.psum_pool(name="pswb", bufs=2) as pswb,
            tc.psum_pool(name="psg", bufs=1) as psg,
        ):
            # resident weights, feature-major lhsT layouts. Small tensors
            # first, then layer-0 weights, then the rest — split into
            # per-layer / per-column-chunk DMAs so the first matmuls only
            # wait on the bytes they actually read.
            xTr = xT.rearrange("(kc p) b -> p kc b", p=P)
            outr = outT.rearrange("(kc p) b -> p kc b", p=P)
            Vlr = Vl.rearrange("i (kc p) m -> p i kc m", p=P)
            Ulr = Ul.rearrange("i (mc p) d -> p i mc d", p=P)
            Cbr = Cb.rearrange("i m p s -> p i m s")

            def load_x0(c, parts=2):
                t = xpool.tile([P, KC, CHUNK], WDT, tag="x0", name=f"x0_{c}")
                cbs = slice(c * CHUNK, (c + 1) * CHUNK)
                step = KC // parts
                for q in range(parts):
                    sl = slice(q * step, (q + 1) * step)
                    nc.sync.dma_start(t[:, sl], xTr[:, sl, cbs])
                return t

            x0_tiles = {0: load_x0(0, parts=4)}
            wg_sb = wpool.tile([P, KC, E], WDT)
            nc.sync.dma_start(wg_sb, WgT.rearrange("(kc p) e -> p kc e", p=P))
            sel_sb = wpool.tile([E, MC + 1, P], WDT)
            nc.sync.dma_start(sel_sb, sel[:])
            b_sb = wpool.tile([P, N_CROSS, KC], F32)
            nc.sync.dma_start(b_sb, bT.rearrange("i p kc -> p i kc"))

            v_sb = wpool.tile([P, N_CROSS, KC, ER], WDT)
            u_sb = wpool.tile([P, N_CROSS, MC, D], WDT)
            c_sb = wpool.tile([P, N_CROSS, MC, P], WDT)

            for i in range(N_CROSS):
                for mc in range(MC):
                    nc.sync.dma_start(
                        v_sb[:, i, :, mc * P : (mc + 1) * P],
                        Vlr[:, i, :, mc * P : (mc + 1) * P],
                    )
                nc.sync.dma_start(c_sb[:, i], Cbr[:, i])
                nc.sync.dma_start(u_sb[:, i], Ulr[:, i])

            for c in range(NCHUNK):
                bs = slice(c * CHUNK, (c + 1) * CHUNK)
                x0 = x0_tiles.pop(c)
                if c + 1 < NCHUNK:
                    x0_tiles[c + 1] = load_x0(c + 1)
                for i in range(N_CROSS):
                    src = x0 if i == 0 else xi
                    # last layer's xi is DMA'd out, so keep it fp32; the
                    # bf16 layers feed the next layer's matmuls.
                    if i == N_CROSS - 1:
                        xi = xipool.tile([P, KC, CHUNK], MMD, tag="xiout")
                    else:
                        xi = xipool.tile([P, KC, CHUNK], WDT, tag="xi")
                    # ---- gating: g[e, b], then w = softmax over e ----
                    gps = psg.tile([E, CHUNK], F32, tag="g")
                    for kc in range(KC):
                        nc.tensor.matmul(
                            gps,
                            wg_sb[:, kc, :],
                            src[:, kc, :],
                            start=(kc == 0),
                            stop=(kc == KC - 1),
                        )
                    expg = spool.tile([E, CHUNK], WDT, tag="expg")
                    nc.scalar.activation(expg, gps, AF.Exp)
                    sums = psg.tile([1, CHUNK], F32, tag="g")
                    nc.tensor.matmul(
                        sums, sel_sb[:, MC, 0:1], expg, start=True, stop=True
                    )
                    rfast = spool.tile([1, CHUNK], F32, tag="rfast")
                    nc.vector.reciprocal_approx_fast(rfast, sums)
                    rrow = spool.tile([1, CHUNK], WDT, tag="rrow")
                    nc.vector.tensor_copy(rrow, rfast)
                    wps = psg.tile([E, CHUNK], F32, tag="g")
                    nc.tensor.matmul(
                        wps, sel_sb[0:1, MC, 0:E], rrow, start=True, stop=True
                    )
                    wsb = spool.tile([E, CHUNK], WDT, tag="wsb")
                    nc.vector.tensor_tensor(wsb, expg, wps, OP.mult)
                    # ---- V stage: h1 = tanh(Vflat.T @ xi) ----
                    h1s = []
                    for mc in range(MC):
                        vps = psmm.tile([P, CHUNK], F32, tag="mm")
                        for kc in range(KC):
                            nc.tensor.matmul(
                                vps,
                                v_sb[:, i, kc, mc * P : (mc + 1) * P],
                                src[:, kc, :],
                                start=(kc == 0),
                                stop=(kc == KC - 1),
                            )
                        h1 = h1pool.tile([P, CHUNK], WDT, tag="h1")
                        nc.scalar.activation(h1, vps, AF.Tanh)
                        h1s.append(h1)
                    # ---- C stage (block-diag expert pairs) + gate scale ----
                    ys = []
                    for mc in range(MC):
                        cps = psmm.tile([P, CHUNK], F32, tag="mm")
                        nc.tensor.matmul(
                            cps, c_sb[:, i, mc, :], h1s[mc], start=True, stop=True
                        )
                        wbp = pswb.tile([P, CHUNK], F32, tag="wb")
                        nc.tensor.matmul(
                            wbp, sel_sb[:, mc, :], wsb, start=True, stop=True
                        )
                        h2 = h2pool.tile([P, CHUNK], F32, tag="h2")
                        nc.scalar.activation(h2, cps, AF.Tanh)
                        y = ypool.tile([P, CHUNK], WDT, tag="y")
                        nc.vector.tensor_tensor(y, h2, wbp, OP.mult)
                        ys.append(y)
                    # ---- U stage + fused combine ----
                    for dc in range(KC):
                        ups = psu.tile([P, CHUNK], F32, tag="u")
                        for mc in range(MC):
                            nc.tensor.matmul(
                                ups,
                                u_sb[:, i, mc, dc * P : (dc + 1) * P],
                                ys[mc],
                                start=(mc == 0),
                                stop=(mc == MC - 1),
                            )
                        tmp = tpool.tile([P, CHUNK], WDT, tag="tmp")
                        nc.vector.scalar_tensor_tensor(
                            tmp,
                            ups,
                            b_sb[:, i, dc : dc + 1],
                            x0[:, dc, :],
                            OP.add,
                            OP.mult,
                        )
                        eng = nc.gpsimd if dc % 2 == 0 else nc.vector
                        eng.tensor_tensor(
                            xi[:, dc, :], tmp, src[:, dc, :], OP.add
                        )
                        if i == N_CROSS - 1:
                            nc.scalar.dma_start(outr[:, dc, bs], xi[:, dc, :])
    nc.compile()
    return nc


_CTX = {}


def _get_nc():
    if "nc" not in _CTX:
        _CTX["nc"] = _build()
    return _CTX["nc"]


def _prep_weights(U, V, C, Wg, b):
    f = np.float32
    U = np.asarray(U, dtype=f)
    V = np.asarray(V, dtype=f)
    C = np.asarray(C, dtype=f)
    Wg = np.asarray(Wg, dtype=f)
    b = np.asarray(b, dtype=f)
    # Vl[i, d, e*R+r] = V[i, e, d, r]
    Vl = np.ascontiguousarray(V.transpose(0, 2, 1, 3).reshape(N_CROSS, D, ER))
    # Ul[i, e*R+r, d] = U[i, e, d, r]
    Ul = np.ascontiguousarray(U.transpose(0, 1, 3, 2).reshape(N_CROSS, ER, D))
    # block-diagonal expert pairs for the C stage
    Cb = np.zeros((N_CROSS, MC, P, P), dtype=f)
    for i in range(N_CROSS):
        for m in range(MC):
            Cb[i, m, :R, :R] = C[i, 2 * m]
            Cb[i, m, R:, R:] = C[i, 2 * m + 1]
    WgT = np.ascontiguousarray(Wg.T)
    # bT[i, p, kc] = b[i, kc*P + p]
    bTa = np.ascontiguousarray(b.reshape(N_CROSS, KC, P).transpose(0, 2, 1))
    # selector planes for broadcasting gate weights over ranks + a ones plane
    sel = np.zeros((E, MC + 1, P), dtype=f)
    for m in range(MC):
        for j in range(P):
            sel[2 * m + j // R, m, j] = 1.0
    sel[:, MC, :] = 1.0
    return dict(
        Vl=Vl.astype(BF16),
        Ul=Ul.astype(BF16),
        Cb=Cb.astype(BF16),
        WgT=WgT.astype(BF16),
        bT=bTa,
        sel=sel.astype(BF16),
    )


def kernel(x, U, V, C, Wg, b, _trace=False):
    nc = _get_nc()
    w = _prep_weights(U, V, C, Wg, b)
    xs = np.asarray(x, dtype=np.float32).reshape(NCORES, BC, D)
    in_maps = []
    for ci in range(NCORES):
        m = {"xT": np.ascontiguousarray(xs[ci].T).astype(BF16)}
        m.update(w)
        in_maps.append(m)
    res = run_bass_kernel_spmd(nc, in_maps, list(range(NCORES)), trace=_trace)
    kernel.last_result = res
    out = np.concatenate(
        [np.asarray(res.results[ci]["outT"]).T for ci in range(NCORES)], axis=0
    )
    return np.ascontiguousarray(out, dtype=np.float32)



# revision 10
# speedup vs baseline: 1.3160x; 1.0170x over previous
"""CrossNetMix (DCN-V2 mixture-of-low-rank-experts) Trainium2 kernel.

Strategy: data-parallel over batch across 8 cores (2048 rows each), with all
tensors kept feature-major on chip ([d, b] layout) so the contraction dim of
every matmul lands on SBUF partitions and no on-chip transposes are needed.
The host pre-transposes each core's x slice and pre-reshapes the weights.

All matmul operands are bf16 (fp32 PSUM accumulation): LDWEIGHTS at 1
cyc/row (~55 ns) fully hides under the 512-row matmuls, unlike fp32r's
224 ns loads which serialize ~55 ns per matmul.

Chunks are processed in interleaved pairs (A, B): per cross layer the
instruction stream is A.gating, A.V, A.C, A.U+combine, then B.* — so B's
matmuls fill the PE while A's combine tail (vector/gpsimd) drains, and
vice versa across layers. Input DMAs are split across both HWDGE queues
(x via Sync/q1, weights via Scalar/q10) in need-order.

Per layer (fused), per chunk:
  g = xi @ Wg.T                  -> gating matmuls, M=8
  w = softmax(g)                 -> exp on ACT; partition-sum / broadcast via
                                    tiny ones/selector matmuls on the PE
                                    (issued after V so PE never waits on exp)
  h1 = tanh(xi @ Vflat)          -> [er=512, b] feature-major
  h2 = tanh(blockdiag_C @ h1)    -> per-expert C folded into 128x128
                                    block-diagonal pairs (4 matmuls)
  y  = h2 * w_broadcast
  mixed = Uflat.T-style matmul   -> [d, b]
  xi = x0 * (mixed + b) + xi     -> stt on vector (PSUM src), add on gpsimd
"""

import os
import sys

import ml_dtypes
import numpy as np

if "/opt/trn_rl_repo" not in sys.path:
    sys.path.insert(0, "/opt/trn_rl_repo")

import concourse.bass as bass
import concourse.bacc as bacc
import concourse.mybir as mybir
from concourse.tile import TileContext
from concourse.bass_utils import run_bass_kernel_spmd

AF = mybir.ActivationFunctionType
OP = mybir.AluOpType
F32 = mybir.dt.float32
WDT = mybir.dt.bfloat16
BF16 = ml_dtypes.bfloat16

N_CROSS = 3
E = 8            # experts
D = 1024         # feature dim
R = 64           # low rank
B = 16384        # full batch
NCORES = 8
BC = B // NCORES  # rows per core
CHUNK = 512       # batch tile (matmul free dim)
NCHUNK = BC // CHUNK
P = 128
KC = D // P       # d-chunks
ER = E * R        # 512
MC = ER // P      # (e,r)-chunks

MMD = mybir.dt.float32r  # output dram dtype (fp32 bits)


def _build():
    nc = bacc.Bacc(None)
    xT = nc.declare_dram_parameter("xT", [D, BC], WDT, isOutput=False)
    Vl = nc.declare_dram_parameter("Vl", [N_CROSS, D, ER], WDT, isOutput=False)
    Cb = nc.declare_dram_parameter("Cb", [N_CROSS, MC, P, P], WDT, isOutput=False)
    Ul = nc.declare_dram_parameter("Ul", [N_CROSS, ER, D], WDT, isOutput=False)
    WgT = nc.declare_dram_parameter("WgT", [D, E], WDT, isOutput=False)
    bT = nc.declare_dram_parameter("bT", [N_CROSS, P, KC], F32, isOutput=False)
    sel = nc.declare_dram_parameter("sel", [E, MC + 1, P], WDT, isOutput=False)
    outT = nc.declare_dram_parameter("outT", [D, BC], MMD, isOutput=True)

    with TileContext(nc) as tc:
        with (
            tc.sbuf_pool(name="wpool", bufs=1) as wpool,
            tc.sbuf_pool(name="xpool", bufs=4) as xpool,
            tc.sbuf_pool(name="xipool", bufs=4) as xipool,
            tc.sbuf_pool(name="xopool", bufs=2) as xopool,
            tc.sbuf_pool(name="h1pool", bufs=4) as h1pool,
            tc.sbuf_pool(name="h2pool", bufs=2) as h2pool,
            tc.sbuf_pool(name="ypool", bufs=5) as ypool,
            tc.sbuf_pool(name="tpool", bufs=3) as tpool,
            tc.sbuf_pool(name="spool", bufs=2) as spool,
            tc.psum_pool(name="psmm", bufs=3) as psmm,
            tc.psum_pool(name="psu", bufs=2) as psu,
            tc.psum_pool(name="pswb", bufs=2) as pswb,
            tc.psum_pool(name="psg", bufs=1) as psg,
        ):
            xTr = xT.rearrange("(kc p) b -> p kc b", p=P)
            outr = outT.rearrange("(kc p) b -> p kc b", p=P)
            Vlr = Vl.rearrange("i (kc p) m -> p i kc m", p=P)
            Ulr = Ul.rearrange("i (mc p) d -> p i mc d", p=P)
            Cbr = Cb.rearrange("i m p s -> p i m s")

            def load_x0(c, parts=1):
                t = xpool.tile([P, KC, CHUNK], WDT, tag="x0", name=f"x0_{c}")
                cbs = slice(c * CHUNK, (c + 1) * CHUNK)
                step = KC // parts
                for q in range(parts):
                    sl = slice(q * step, (q + 1) * step)
                    nc.sync.dma_start(t[:, sl], xTr[:, sl, cbs])
                return t

            # x loads on the Sync HWDGE queue; weights on the Scalar HWDGE
            # queue so the two streams transfer concurrently, each in
            # need-order.
            x0_tiles = {0: load_x0(0, parts=2), 1: load_x0(1)}

            wg_sb = wpool.tile([P, KC, E], WDT)
            nc.scalar.dma_start(wg_sb, WgT.rearrange("(kc p) e -> p kc e", p=P))
            sel_sb = wpool.tile([E, MC + 1, P], WDT)
            nc.scalar.dma_start(sel_sb, sel[:])

            v_sb = wpool.tile([P, N_CROSS, KC, ER], WDT)
            u_sb = wpool.tile([P, N_CROSS, MC, D], WDT)
            c_sb = wpool.tile([P, N_CROSS, MC, P], WDT)
            b_sb = wpool.tile([P, N_CROSS, KC], F32)

            # layer-0 V first (first big weight the PE needs), split for
            # earlier first-matmul; then U0, bias, all C, then layers 1-2.
            nc.scalar.dma_start(v_sb[:, 0, :, 0:ER // 2], Vlr[:, 0, :, 0:ER // 2])
            nc.scalar.dma_start(v_sb[:, 0, :, ER // 2:], Vlr[:, 0, :, ER // 2:])
            nc.scalar.dma_start(u_sb[:, 0], Ulr[:, 0])
            nc.scalar.dma_start(b_sb, bT.rearrange("i p kc -> p i kc"))
            nc.scalar.dma_start(c_sb, Cbr)
            nc.scalar.dma_start(v_sb[:, 1], Vlr[:, 1])
            nc.scalar.dma_start(u_sb[:, 1], Ulr[:, 1])
            nc.scalar.dma_start(v_sb[:, 2], Vlr[:, 2])
            nc.scalar.dma_start(u_sb[:, 2], Ulr[:, 2])

            xi_tiles = {}
            for pair in range(NCHUNK // 2):
                for i in range(N_CROSS):
                    for half in range(2):
                        c = 2 * pair + half
                        # prefetch next pair's x during layer 1
                        if i == 1 and c + 2 < NCHUNK:
                            x0_tiles[c + 2] = load_x0(c + 2)
                        bs = slice(c * CHUNK, (c + 1) * CHUNK)
                        x0 = x0_tiles[c]
                        src = x0 if i == 0 else xi_tiles.pop(c)
                        if i == N_CROSS - 1:
                            xi = xopool.tile([P, KC, CHUNK], MMD, tag="xiout")
                            x0_tiles.pop(c)
                        else:
                            xi = xipool.tile([P, KC, CHUNK], WDT, tag="xi")
                            xi_tiles[c] = xi
                        # ---- gating matmuls ----
                        gps = psg.tile([E, CHUNK], F32, tag="g")
                        for kc in range(KC):
                            nc.tensor.matmul(
                                gps,
                                wg_sb[:, kc, :],
                                src[:, kc, :],
                                start=(kc == 0),
                                stop=(kc == KC - 1),
                            )
                        expg = spool.tile([E, CHUNK], WDT, tag="expg")
                        nc.scalar.activation(expg, gps, AF.Exp)
                        # ---- V stage: h1 = tanh(Vflat.T @ xi) ----
                        h1s = []
                        for mc in range(MC):
                            vps = psmm.tile([P, CHUNK], F32, tag="mm")
                            for kc in range(KC):
                                nc.tensor.matmul(
                                    vps,
                                    v_sb[:, i, kc, mc * P : (mc + 1) * P],
                                    src[:, kc, :],
                                    start=(kc == 0),
                                    stop=(kc == KC - 1),
                                )
                            h1 = h1pool.tile([P, CHUNK], WDT, tag="h1")
                            nc.scalar.activation(h1, vps, AF.Tanh)
                            h1s.append(h1)
                        # ---- softmax tail (PE parts after V so the PE
                        # never waits on the exp activation) ----
                        sums = psg.tile([1, CHUNK], F32, tag="g")
                        nc.tensor.matmul(
                            sums, sel_sb[:, MC, 0:1], expg, start=True, stop=True
                        )
                        rfast = spool.tile([1, CHUNK], F32, tag="rfast")
                        nc.vector.reciprocal_approx_fast(rfast, sums)
                        rrow = spool.tile([1, CHUNK], WDT, tag="rrow")
                        nc.vector.tensor_copy(rrow, rfast)
                        wps = psg.tile([E, CHUNK], F32, tag="g")
                        nc.tensor.matmul(
                            wps, sel_sb[0:1, MC, 0:E], rrow, start=True, stop=True
                        )
                        wsb = spool.tile([E, CHUNK], WDT, tag="wsb")
                        nc.vector.tensor_tensor(wsb, expg, wps, OP.mult)
                        # ---- C stage (block-diag expert pairs) + gate ----
                        ys = []
                        for mc in range(MC):
                            cps = psmm.tile([P, CHUNK], F32, tag="mm")
                            nc.tensor.matmul(
                                cps, c_sb[:, i, mc, :], h1s[mc], start=True, stop=True
                            )
                            wbp = pswb.tile([P, CHUNK], F32, tag="wb")
                            nc.tensor.matmul(
                                wbp, sel_sb[:, mc, :], wsb, start=True, stop=True
                            )
                            h2 = h2pool.tile([P, CHUNK], F32, tag="h2")
                            nc.scalar.activation(h2, cps, AF.Tanh)
                            y = ypool.tile([P, CHUNK], WDT, tag="y")
                            nc.vector.tensor_tensor(y, h2, wbp, OP.mult)
                            ys.append(y)
                        # ---- U stage + fused combine ----
                        for dc in range(KC):
                            ups = psu.tile([P, CHUNK], F32, tag="u")
                            for mc in range(MC):
                                nc.tensor.matmul(
                                    ups,
                                    u_sb[:, i, mc, dc * P : (dc + 1) * P],
                                    ys[mc],
                                    start=(mc == 0),
                                    stop=(mc == MC - 1),
                                )
                            tmp = tpool.tile([P, CHUNK], WDT, tag="tmp")
                            nc.vector.scalar_tensor_tensor(
                                tmp,
                                ups,
                                b_sb[:, i, dc : dc + 1],
                                x0[:, dc, :],
                                OP.add,
                                OP.mult,
                            )
                            nc.gpsimd.tensor_tensor(
                                xi[:, dc, :], tmp, src[:, dc, :], OP.add
                            )
                            if i == N_CROSS - 1:
                                eng = nc.sync if dc % 2 == 0 else nc.scalar
                                eng.dma_start(outr[:, dc, bs], xi[:, dc, :])
    nc.compile()
    return nc


_CTX = {}


def _get_nc():
    if "nc" not in _CTX:
        _CTX["nc"] = _build()
    return _CTX["nc"]


def _prep_weights(U, V, C, Wg, b):
    f = np.float32
    U = np.asarray(U, dtype=f)
    V = np.asarray(V, dtype=f)
    C = np.asarray(C, dtype=f)
    Wg = np.asarray(Wg, dtype=f)
    b = np.asarray(b, dtype=f)
    # Vl[i, d, e*R+r] = V[i, e, d, r]
    Vl = np.ascontiguousarray(V.transpose(0, 2, 1, 3).reshape(N_CROSS, D, ER))
    # Ul[i, e*R+r, d] = U[i, e, d, r]
    Ul = np.ascontiguousarray(U.transpose(0, 1, 3, 2).reshape(N_CROSS, ER, D))
    # block-diagonal expert pairs for the C stage
    Cb = np.zeros((N_CROSS, MC, P, P), dtype=f)
    for i in range(N_CROSS):
        for m in range(MC):
            Cb[i, m, :R, :R] = C[i, 2 * m]
            Cb[i, m, R:, R:] = C[i, 2 * m + 1]
    WgT = np.ascontiguousarray(Wg.T)
    # bT[i, p, kc] = b[i, kc*P + p]
    bTa = np.ascontiguousarray(b.reshape(N_CROSS, KC, P).transpose(0, 2, 1))
    # selector planes for broadcasting gate weights over ranks + a ones plane
    sel = np.zeros((E, MC + 1, P), dtype=f)
    for m in range(MC):
        for j in range(P):
            sel[2 * m + j // R, m, j] = 1.0
    sel[:, MC, :] = 1.0
    return dict(
        Vl=Vl.astype(BF16),
        Ul=Ul.astype(BF16),
        Cb=Cb.astype(BF16),
        WgT=WgT.astype(BF16),
        bT=bTa,
        sel=sel.astype(BF16),
    )


def kernel(x, U, V, C, Wg, b, _trace=False):
    nc = _get_nc()
    w = _prep_weights(U, V, C, Wg, b)
    xs = np.asarray(x, dtype=np.float32).reshape(NCORES, BC, D)
    in_maps = []
    for ci in range(NCORES):
        m = {"xT": np.ascontiguousarray(xs[ci].T).astype(BF16)}
        m.update(w)
        in_maps.append(m)
    res = run_bass_kernel_spmd(nc, in_maps, list(range(NCORES)), trace=_trace)
    kernel.last_result = res
    out = np.concatenate(
        [np.asarray(res.results[ci]["outT"]).T for ci in range(NCORES)], axis=0
    )
    return np.ascontiguousarray(out, dtype=np.float32)


# revision 14
# speedup vs baseline: 1.3794x; 1.0482x over previous
"""CrossNetMix (DCN-V2 mixture-of-low-rank-experts) Trainium2 kernel.

Strategy: data-parallel over batch across 8 cores (2048 rows each), with all
tensors kept feature-major on chip ([d, b] layout) so the contraction dim of
every matmul lands on SBUF partitions and no on-chip transposes are needed.
The host pre-transposes each core's x slice and pre-reshapes the weights.

All matmul operands are bf16 (fp32 PSUM accumulation): LDWEIGHTS at 1
cyc/row (~55 ns) fully hides under the 512-row matmuls, unlike fp32r's
224 ns loads which serialize ~55 ns per matmul.

Chunks are processed in interleaved pairs (A, B): per cross layer the
instruction stream is A.gating, A.V, A.C, A.U+combine, then B.* — so B's
matmuls fill the PE while A's combine tail (vector/gpsimd) drains, and
vice versa across layers. Input DMAs are split across both HWDGE queues
(x via Sync/q1, weights via Scalar/q10) in need-order.

Per layer (fused), per chunk:
  g = xi @ Wg.T                  -> gating matmuls, M=8
  w = softmax(g)                 -> exp on ACT; partition-sum / broadcast via
                                    tiny ones/selector matmuls on the PE
                                    (issued after V so PE never waits on exp)
  h1 = tanh(xi @ Vflat)          -> [er=512, b] feature-major
  h2 = tanh(blockdiag_C @ h1)    -> per-expert C folded into 128x128
                                    block-diagonal pairs (4 matmuls)
  y  = h2 * w_broadcast
  mixed = Uflat.T-style matmul   -> [d, b]
  xi = x0 * (mixed + b) + xi     -> stt on vector (PSUM src), add on gpsimd
"""

import os
import sys

import ml_dtypes
import numpy as np

if "/opt/trn_rl_repo" not in sys.path:
    sys.path.insert(0, "/opt/trn_rl_repo")

import concourse.bass as bass
import concourse.bacc as bacc
import concourse.mybir as mybir
from concourse.tile import TileContext
from concourse.bass_utils import run_bass_kernel_spmd

AF = mybir.ActivationFunctionType
OP = mybir.AluOpType
F32 = mybir.dt.float32
WDT = mybir.dt.bfloat16
BF16 = ml_dtypes.bfloat16

N_CROSS = 3
E = 8            # experts
D = 1024         # feature dim
R = 64           # low rank
B = 16384        # full batch
NCORES = 8
BC = B // NCORES  # rows per core
CHUNK = 512       # batch tile (matmul free dim)
NCHUNK = BC // CHUNK
P = 128
KC = D // P       # d-chunks
ER = E * R        # 512
MC = ER // P      # (e,r)-chunks

MMD = mybir.dt.float32r  # output dram dtype (fp32 bits)


def _build():
    nc = bacc.Bacc(None)
    xT = nc.declare_dram_parameter("xT", [D, BC], WDT, isOutput=False)
    Vl = nc.declare_dram_parameter("Vl", [N_CROSS, D, ER], WDT, isOutput=False)
    Cb = nc.declare_dram_parameter("Cb", [N_CROSS, MC, P, P], WDT, isOutput=False)
    Ul = nc.declare_dram_parameter("Ul", [N_CROSS, ER, D], WDT, isOutput=False)
    WgT = nc.declare_dram_parameter("WgT", [D, E], WDT, isOutput=False)
    bT = nc.declare_dram_parameter("bT", [N_CROSS, P, KC], F32, isOutput=False)
    sel = nc.declare_dram_parameter("sel", [E, MC + 1, P], WDT, isOutput=False)
    outT = nc.declare_dram_parameter("outT", [D, BC], WDT, isOutput=True)

    with TileContext(nc) as tc:
        with (
            tc.sbuf_pool(name="wpool", bufs=1) as wpool,
            tc.sbuf_pool(name="xpool", bufs=4) as xpool,
            tc.sbuf_pool(name="xipool", bufs=4) as xipool,
            tc.sbuf_pool(name="xopool", bufs=2) as xopool,
            tc.sbuf_pool(name="h1pool", bufs=4) as h1pool,
            tc.sbuf_pool(name="h2pool", bufs=2) as h2pool,
            tc.sbuf_pool(name="ypool", bufs=5) as ypool,
            tc.sbuf_pool(name="tpool", bufs=3) as tpool,
            tc.sbuf_pool(name="spool", bufs=2) as spool,
            tc.psum_pool(name="psmm", bufs=3) as psmm,
            tc.psum_pool(name="psu", bufs=2) as psu,
            tc.psum_pool(name="pswb", bufs=2) as pswb,
            tc.psum_pool(name="psg", bufs=1) as psg,
        ):
            xTr = xT.rearrange("(kc p) b -> p kc b", p=P)
            outr = outT.rearrange("(kc p) b -> p kc b", p=P)
            Vlr = Vl.rearrange("i (kc p) m -> p i kc m", p=P)
            Ulr = Ul.rearrange("i (mc p) d -> p i mc d", p=P)
            Cbr = Cb.rearrange("i m p s -> p i m s")

            def load_x0(c, parts=1):
                t = xpool.tile([P, KC, CHUNK], WDT, tag="x0", name=f"x0_{c}")
                cbs = slice(c * CHUNK, (c + 1) * CHUNK)
                step = KC // parts
                for q in range(parts):
                    sl = slice(q * step, (q + 1) * step)
                    nc.sync.dma_start(t[:, sl], xTr[:, sl, cbs])
                return t

            # Split input loads across both HWDGE queues in need-order so
            # the two streams transfer concurrently:
            #   Sync/q1:   x chunks, then C + U0/U1 (needed ~25-60us in)
            #   Scalar/q10: gating weights + per-layer V, then the rest
            wg_sb = wpool.tile([P, KC, E], WDT)
            nc.scalar.dma_start(wg_sb, WgT.rearrange("(kc p) e -> p kc e", p=P))

            v_sb = wpool.tile([P, N_CROSS, KC, ER], WDT)
            u_sb = wpool.tile([P, N_CROSS, MC, D], WDT)
            c_sb = wpool.tile([P, N_CROSS, MC, P], WDT)
            b_sb = wpool.tile([P, N_CROSS, KC], F32)

            # layer-0 V split along kc (keeps 1KB-contiguous rows per packet)
            nc.scalar.dma_start(v_sb[:, 0, 0:KC // 2], Vlr[:, 0, 0:KC // 2])
            nc.scalar.dma_start(v_sb[:, 0, KC // 2:], Vlr[:, 0, KC // 2:])

            x0_tiles = {0: load_x0(0, parts=2), 1: load_x0(1)}
            nc.sync.dma_start(c_sb, Cbr)
            nc.sync.dma_start(u_sb[:, 0], Ulr[:, 0])
            nc.sync.dma_start(u_sb[:, 1], Ulr[:, 1])

            sel_sb = wpool.tile([E, MC + 1, P], WDT)
            nc.scalar.dma_start(sel_sb, sel[:])
            nc.scalar.dma_start(v_sb[:, 1], Vlr[:, 1])
            nc.scalar.dma_start(b_sb, bT.rearrange("i p kc -> p i kc"))
            nc.scalar.dma_start(v_sb[:, 2], Vlr[:, 2])
            nc.scalar.dma_start(u_sb[:, 2], Ulr[:, 2])

            xi_tiles = {}
            for pair in range(NCHUNK // 2):
                for i in range(N_CROSS):
                    for half in range(2):
                        c = 2 * pair + half
                        # prefetch next pair's x during layer 1
                        if i == 1 and c + 2 < NCHUNK:
                            x0_tiles[c + 2] = load_x0(c + 2)
                        bs = slice(c * CHUNK, (c + 1) * CHUNK)
                        x0 = x0_tiles[c]
                        src = x0 if i == 0 else xi_tiles.pop(c)
                        if i == N_CROSS - 1:
                            xi = xopool.tile([P, KC, CHUNK], WDT, tag="xiout")
                            x0_tiles.pop(c)
                        else:
                            xi = xipool.tile([P, KC, CHUNK], WDT, tag="xi")
                            xi_tiles[c] = xi
                        # ---- gating matmuls ----
                        gps = psg.tile([E, CHUNK], F32, tag="g")
                        for kc in range(KC):
                            nc.tensor.matmul(
                                gps,
                                wg_sb[:, kc, :],
                                src[:, kc, :],
                                start=(kc == 0),
                                stop=(kc == KC - 1),
                            )
                        expg = spool.tile([E, CHUNK], WDT, tag="expg")
                        nc.scalar.activation(expg, gps, AF.Exp)
                        # ---- V stage: h1 = tanh(Vflat.T @ xi) ----
                        h1s = []
                        for mc in range(MC):
                            vps = psmm.tile([P, CHUNK], F32, tag="mm")
                            for kc in range(KC):
                                nc.tensor.matmul(
                                    vps,
                                    v_sb[:, i, kc, mc * P : (mc + 1) * P],
                                    src[:, kc, :],
                                    start=(kc == 0),
                                    stop=(kc == KC - 1),
                                )
                            h1 = h1pool.tile([P, CHUNK], WDT, tag="h1")
                            nc.scalar.activation(h1, vps, AF.Tanh)
                            h1s.append(h1)
                        # ---- softmax tail (PE parts after V so the PE
                        # never waits on the exp activation) ----
                        sums = psg.tile([1, CHUNK], F32, tag="g")
                        nc.tensor.matmul(
                            sums, sel_sb[:, MC, 0:1], expg, start=True, stop=True
                        )
                        rfast = spool.tile([1, CHUNK], F32, tag="rfast")
                        nc.vector.reciprocal_approx_fast(rfast, sums)
                        rrow = spool.tile([1, CHUNK], WDT, tag="rrow")
                        nc.vector.tensor_copy(rrow, rfast)
                        wps = psg.tile([E, CHUNK], F32, tag="g")
                        nc.tensor.matmul(
                            wps, sel_sb[0:1, MC, 0:E], rrow, start=True, stop=True
                        )
                        wsb = spool.tile([E, CHUNK], WDT, tag="wsb")
                        nc.vector.tensor_tensor(wsb, expg, wps, OP.mult)
                        # ---- C stage (block-diag expert pairs) + gate ----
                        ys = []
                        for mc in range(MC):
                            cps = psmm.tile([P, CHUNK], F32, tag="mm")
                            nc.tensor.matmul(
                                cps, c_sb[:, i, mc, :], h1s[mc], start=True, stop=True
                            )
                            wbp = pswb.tile([P, CHUNK], F32, tag="wb")
                            nc.tensor.matmul(
                                wbp, sel_sb[:, mc, :], wsb, start=True, stop=True
                            )
                            h2 = h2pool.tile([P, CHUNK], F32, tag="h2")
                            nc.scalar.activation(h2, cps, AF.Tanh)
                            y = ypool.tile([P, CHUNK], WDT, tag="y")
                            nc.vector.tensor_tensor(y, h2, wbp, OP.mult)
                            ys.append(y)
                        # ---- U stage + fused combine ----
                        for dc in range(KC):
                            ups = psu.tile([P, CHUNK], F32, tag="u")
                            for mc in range(MC):
                                nc.tensor.matmul(
                                    ups,
                                    u_sb[:, i, mc, dc * P : (dc + 1) * P],
                                    ys[mc],
                                    start=(mc == 0),
                                    stop=(mc == MC - 1),
                                )
                            tmp = tpool.tile([P, CHUNK], WDT, tag="tmp")
                            nc.vector.scalar_tensor_tensor(
                                tmp,
                                ups,
                                b_sb[:, i, dc : dc + 1],
                                x0[:, dc, :],
                                OP.add,
                                OP.mult,
                            )
                            nc.gpsimd.tensor_tensor(
                                xi[:, dc, :], tmp, src[:, dc, :], OP.add
                            )
                            if i == N_CROSS - 1:
                                eng = nc.sync if dc % 2 == 0 else nc.scalar
                                eng.dma_start(outr[:, dc, bs], xi[:, dc, :])
    nc.compile()
    return nc


_CTX = {}


def _get_nc():
    if "nc" not in _CTX:
        _CTX["nc"] = _build()
    return _CTX["nc"]


def _prep_weights(U, V, C, Wg, b):
    f = np.float32
    U = np.asarray(U, dtype=f)
    V = np.asarray(V, dtype=f)
    C = np.asarray(C, dtype=f)
    Wg = np.asarray(Wg, dtype=f)
    b = np.asarray(b, dtype=f)
    # Vl[i, d, e*R+r] = V[i, e, d, r]
    Vl = np.ascontiguousarray(V.transpose(0, 2, 1, 3).reshape(N_CROSS, D, ER))
    # Ul[i, e*R+r, d] = U[i, e, d, r]
    Ul = np.ascontiguousarray(U.transpose(0, 1, 3, 2).reshape(N_CROSS, ER, D))
    # block-diagonal expert pairs for the C stage
    Cb = np.zeros((N_CROSS, MC, P, P), dtype=f)
    for i in range(N_CROSS):
        for m in range(MC):
            Cb[i, m, :R, :R] = C[i, 2 * m]
            Cb[i, m, R:, R:] = C[i, 2 * m + 1]
    WgT = np.ascontiguousarray(Wg.T)
    # bT[i, p, kc] = b[i, kc*P + p]
    bTa = np.ascontiguousarray(b.reshape(N_CROSS, KC, P).transpose(0, 2, 1))
    # selector planes for broadcasting gate weights over ranks + a ones plane
    sel = np.zeros((E, MC + 1, P), dtype=f)
    for m in range(MC):
        for j in range(P):
            sel[2 * m + j // R, m, j] = 1.0
    sel[:, MC, :] = 1.0
    return dict(
        Vl=Vl.astype(BF16),
        Ul=Ul.astype(BF16),
        Cb=Cb.astype(BF16),
        WgT=WgT.astype(BF16),
        bT=bTa,
        sel=sel.astype(BF16),
    )


def kernel(x, U, V, C, Wg, b, _trace=False):
    nc = _get_nc()
    w = _prep_weights(U, V, C, Wg, b)
    xs = np.asarray(x, dtype=np.float32).reshape(NCORES, BC, D)
    in_maps = []
    for ci in range(NCORES):
        m = {"xT": np.ascontiguousarray(xs[ci].T).astype(BF16)}
        m.update(w)
        in_maps.append(m)
    res = run_bass_kernel_spmd(nc, in_maps, list(range(NCORES)), trace=_trace)
    kernel.last_result = res
    out = np.concatenate(
        [np.asarray(res.results[ci]["outT"]).astype(np.float32).T
         for ci in range(NCORES)],
        axis=0,
    )
    return np.ascontiguousarray(out, dtype=np.float32)


# revision 20
# speedup vs baseline: 1.7282x; 1.2528x over previous
"""CrossNetMix (DCN-V2 mixture-of-low-rank-experts) Trainium2 kernel.

Data-parallel over batch across 8 cores (2048 rows each); feature-major
([d, b]) on chip so every matmul contraction lands on SBUF partitions.

Matmul stages (gating, V, C, U) run in fp8-e4m3 DoubleRow mode: each
matmul contracts two adjacent 128-row k-subtiles per pass (2 rows/cycle),
roughly halving tensor-engine time vs bf16. PSUM accumulates in fp32.
The softmax/gate-broadcast helper matmuls stay bf16.

Residual reformulation: with S_i = sum_{j<i} (uv_j + b_j),
  xi_i = x0 ⊙ (S_i + 1) .
Each layer's U matmuls accumulate uv into PSUM; for layers > 0 an identity
matmul adds the previous S (bf16, SBUF) into the same accumulation. Then
one scalar_tensor_tensor per d-chunk emits xi = (S + (1 + B_i)) ⊙ x0
directly in fp8 for the next layer's matmuls (bf16 on the last layer for
the output DMA), with the bias cumsum B_i folded into the per-partition
scalar. An ACT copy spills S back to SBUF for the next layer. This keeps
the whole combine at ~2 elementwise ops per d-chunk with no bf16 residual
round-trip on x0.

Chunks are processed in interleaved pairs (A, B): per layer the stream is
A.gating, A.V, A.C, A.U+combine then B.*, so B's matmuls cover A's
combine/softmax tails. Input DMAs are split across both HWDGE queues
(x via Sync/q1, V weights via Scalar/q10) in need-order.
"""

import os
import sys

import ml_dtypes
import numpy as np

if "/opt/trn_rl_repo" not in sys.path:
    sys.path.insert(0, "/opt/trn_rl_repo")

import concourse.bass as bass
import concourse.bacc as bacc
import concourse.mybir as mybir
from concourse.tile import TileContext
from concourse.bass_utils import run_bass_kernel_spmd

AF = mybir.ActivationFunctionType
OP = mybir.AluOpType
DR = mybir.MatmulPerfMode.DoubleRow
F32 = mybir.dt.float32
WDT = mybir.dt.bfloat16
F8 = mybir.dt.float8e4
BF16 = ml_dtypes.bfloat16
NPF8 = ml_dtypes.float8_e4m3

N_CROSS = 3
E = 8            # experts
D = 1024         # feature dim
R = 64           # low rank
B = 16384        # full batch
NCORES = 8
BC = B // NCORES  # rows per core
CHUNK = 512       # batch tile (matmul free dim)
NCHUNK = BC // CHUNK
P = 128
KC = D // P       # d-chunks
ER = E * R        # 512
MC = ER // P      # (e,r)-chunks


def _build():
    nc = bacc.Bacc(None)
    xT = nc.declare_dram_parameter("xT", [D, BC], WDT, isOutput=False)
    xT8 = nc.declare_dram_parameter("xT8", [D, BC], F8, isOutput=False)
    Vl = nc.declare_dram_parameter("Vl", [N_CROSS, D, ER], F8, isOutput=False)
    Cb = nc.declare_dram_parameter("Cb", [N_CROSS, MC, 2, P, P], F8, isOutput=False)
    Ul = nc.declare_dram_parameter("Ul", [N_CROSS, ER, D], F8, isOutput=False)
    # gating weights padded to 16 cols: DoubleRow lhsT outer step must be
    # a multiple of 16 (s3_lw dual-fp8 restriction)
    WgT = nc.declare_dram_parameter("WgT", [D, 2 * E], F8, isOutput=False)
    # bTc[i, p, kc] = 1 + sum_{j<=i} b[j, kc*P+p]  (per-partition stt scalar)
    bTc = nc.declare_dram_parameter("bTc", [N_CROSS, P, KC], F32, isOutput=False)
    sel = nc.declare_dram_parameter("sel", [E, MC + 1, P], WDT, isOutput=False)
    id128 = nc.declare_dram_parameter("id128", [P, P], WDT, isOutput=False)
    outT = nc.declare_dram_parameter("outT", [D, BC], WDT, isOutput=True)

    with TileContext(nc) as tc:
        with (
            tc.sbuf_pool(name="wpool", bufs=1) as wpool,
            tc.sbuf_pool(name="xpool", bufs=4) as xpool,
            tc.sbuf_pool(name="x8pool", bufs=3) as x8pool,
            tc.sbuf_pool(name="xipool", bufs=4) as xipool,
            tc.sbuf_pool(name="spool2", bufs=4) as spool2,
            tc.sbuf_pool(name="xopool", bufs=2) as xopool,
            tc.sbuf_pool(name="h1pool", bufs=2) as h1pool,
            tc.sbuf_pool(name="h2pool", bufs=2) as h2pool,
            tc.sbuf_pool(name="ypool", bufs=2) as ypool,
            tc.sbuf_pool(name="spool", bufs=2) as spool,
            tc.psum_pool(name="psmm", bufs=3) as psmm,
            tc.psum_pool(name="psu", bufs=2) as psu,
            tc.psum_pool(name="pswb", bufs=2) as pswb,
            tc.psum_pool(name="psg", bufs=1) as psg,
        ):
            xTr = xT.rearrange("(kc p) b -> p kc b", p=P)
            xT8r = xT8.rearrange("(kc p) b -> p kc b", p=P)
            outr = outT.rearrange("(kc p) b -> p kc b", p=P)
            Vlr = Vl.rearrange("i (kc p) m -> p i kc m", p=P)
            Ulr = Ul.rearrange("i (mc p) d -> p i mc d", p=P)
            Cbr = Cb.rearrange("i m j p s -> p i m j s")

            def load_x0(c, parts=1):
                t = xpool.tile([P, KC, CHUNK], WDT, tag="x0", name=f"x0_{c}")
                t8 = x8pool.tile([P, KC, CHUNK], F8, tag="x08", name=f"x08_{c}")
                cbs = slice(c * CHUNK, (c + 1) * CHUNK)
                step = KC // parts
                for q in range(parts):
                    sl = slice(q * step, (q + 1) * step)
                    nc.sync.dma_start(t8[:, sl], xT8r[:, sl, cbs])
                for q in range(parts):
                    sl = slice(q * step, (q + 1) * step)
                    nc.sync.dma_start(t[:, sl], xTr[:, sl, cbs])
                return t, t8

            wg_sb = wpool.tile([P, KC, 2 * E], F8)
            nc.scalar.dma_start(wg_sb, WgT.rearrange("(kc p) e -> p kc e", p=P))

            v_sb = wpool.tile([P, N_CROSS, KC, ER], F8)
            u_sb = wpool.tile([P, N_CROSS, MC, D], F8)
            c_sb = wpool.tile([P, N_CROSS, MC, 2, P], F8)
            b_sb = wpool.tile([P, N_CROSS, KC], F32)
            id_sb = wpool.tile([P, P], WDT)

            nc.scalar.dma_start(v_sb[:, 0, 0:KC // 2], Vlr[:, 0, 0:KC // 2])
            nc.scalar.dma_start(v_sb[:, 0, KC // 2:], Vlr[:, 0, KC // 2:])

            x0_tiles = {0: load_x0(0, parts=2), 1: load_x0(1)}
            nc.sync.dma_start(c_sb, Cbr)
            nc.sync.dma_start(id_sb, id128[:])
            nc.sync.dma_start(u_sb[:, 0], Ulr[:, 0])
            nc.sync.dma_start(u_sb[:, 1], Ulr[:, 1])

            sel_sb = wpool.tile([E, MC + 1, P], WDT)
            nc.scalar.dma_start(sel_sb, sel[:])
            nc.scalar.dma_start(v_sb[:, 1], Vlr[:, 1])
            nc.scalar.dma_start(b_sb, bTc.rearrange("i p kc -> p i kc"))
            nc.scalar.dma_start(v_sb[:, 2], Vlr[:, 2])
            nc.scalar.dma_start(u_sb[:, 2], Ulr[:, 2])

            s_tiles = {}
            xi8_tiles = {}
            for pair in range(NCHUNK // 2):
                for i in range(N_CROSS):
                    for half in range(2):
                        c = 2 * pair + half
                        if i == 1 and c + 2 < NCHUNK:
                            x0_tiles[c + 2] = load_x0(c + 2)
                        bs = slice(c * CHUNK, (c + 1) * CHUNK)
                        x0, x08 = x0_tiles[c]
                        src8 = x08 if i == 0 else xi8_tiles.pop(c)
                        s_prev = None if i == 0 else s_tiles.pop(c)
                        last = i == N_CROSS - 1
                        if last:
                            xi = xopool.tile([P, KC, CHUNK], WDT, tag="xiout")
                            x0_tiles.pop(c)
                        else:
                            xi = xipool.tile([P, KC, CHUNK], F8, tag="xi8")
                            xi8_tiles[c] = xi
                            s_new = spool2.tile([P, KC, CHUNK], WDT, tag="s")
                            s_tiles[c] = s_new
                        # ---- gating (fp8 DoubleRow, 16-col padded) ----
                        gps = psg.tile([2 * E, CHUNK], F32, tag="g")
                        for q in range(KC // 2):
                            nc.tensor.matmul(
                                gps,
                                wg_sb[:, 2 * q : 2 * q + 2, :],
                                src8[:, 2 * q : 2 * q + 2, :],
                                start=(q == 0),
                                stop=(q == KC // 2 - 1),
                                perf_mode=DR,
                            )
                        expg = spool.tile([E, CHUNK], WDT, tag="expg")
                        nc.scalar.activation(expg, gps[0:E, :], AF.Exp)
                        # ---- V stage (fp8 DoubleRow) ----
                        h1 = h1pool.tile([P, MC, CHUNK], F8, tag="h1")
                        for mc in range(MC):
                            vps = psmm.tile([P, CHUNK], F32, tag="mm")
                            for q in range(KC // 2):
                                nc.tensor.matmul(
                                    vps,
                                    v_sb[:, i, 2 * q : 2 * q + 2,
                                         mc * P : (mc + 1) * P],
                                    src8[:, 2 * q : 2 * q + 2, :],
                                    start=(q == 0),
                                    stop=(q == KC // 2 - 1),
                                    perf_mode=DR,
                                )
                            nc.scalar.activation(h1[:, mc, :], vps, AF.Tanh)
                        # ---- softmax tail ----
                        sums = psg.tile([1, CHUNK], F32, tag="g")
                        nc.tensor.matmul(
                            sums, sel_sb[:, MC, 0:1], expg, start=True, stop=True
                        )
                        rfast = spool.tile([1, CHUNK], F32, tag="rfast")
                        nc.vector.reciprocal_approx_fast(rfast, sums)
                        rrow = spool.tile([1, CHUNK], WDT, tag="rrow")
                        nc.gpsimd.tensor_copy(rrow, rfast)
                        wps = psg.tile([E, CHUNK], F32, tag="g")
                        nc.tensor.matmul(
                            wps, sel_sb[0:1, MC, 0:E], rrow, start=True, stop=True
                        )
                        wsb = spool.tile([E, CHUNK], WDT, tag="wsb")
                        nc.vector.tensor_tensor(wsb, expg, wps, OP.mult)
                        # ---- C stage (fp8 DoubleRow, zero-padded pair) ----
                        ys = ypool.tile([P, MC, CHUNK], F8, tag="y")
                        for mc in range(MC):
                            cps = psmm.tile([P, CHUNK], F32, tag="mm")
                            qb = (mc // 2) * 2
                            nc.tensor.matmul(
                                cps,
                                c_sb[:, i, mc, :, :],
                                h1[:, qb : qb + 2, :],
                                start=True,
                                stop=True,
                                perf_mode=DR,
                            )
                            wbp = pswb.tile([P, CHUNK], F32, tag="wb")
                            nc.tensor.matmul(
                                wbp, sel_sb[:, mc, :], wsb, start=True, stop=True
                            )
                            h2 = h2pool.tile([P, CHUNK], F32, tag="h2")
                            nc.scalar.activation(h2, cps, AF.Tanh)
                            nc.vector.tensor_tensor(ys[:, mc, :], h2, wbp, OP.mult)
                        # ---- U stage + S accumulate + combine ----
                        for dc in range(KC):
                            ups = psu.tile([P, CHUNK], F32, tag="u")
                            for q in range(MC // 2):
                                nc.tensor.matmul(
                                    ups,
                                    u_sb[:, i, 2 * q : 2 * q + 2,
                                         dc * P : (dc + 1) * P],
                                    ys[:, 2 * q : 2 * q + 2, :],
                                    start=(q == 0),
                                    stop=(s_prev is None and q == MC // 2 - 1),
                                    perf_mode=DR,
                                )
                            if s_prev is not None:
                                nc.tensor.matmul(
                                    ups, id_sb, s_prev[:, dc, :],
                                    start=False, stop=True,
                                )
                            # xi = (S + (1 + B_i)) * x0
                            nc.vector.scalar_tensor_tensor(
                                xi[:, dc, :],
                                ups,
                                b_sb[:, i, dc : dc + 1],
                                x0[:, dc, :],
                                OP.add,
                                OP.mult,
                            )
                            if not last:
                                nc.scalar.activation(
                                    s_new[:, dc, :], ups, AF.Copy
                                )
                            else:
                                eng = nc.sync if dc % 2 == 0 else nc.scalar
                                eng.dma_start(outr[:, dc, bs], xi[:, dc, :])
    nc.compile()
    return nc


_CTX = {}


def _get_nc():
    if "nc" not in _CTX:
        _CTX["nc"] = _build()
    return _CTX["nc"]


def _prep_weights(U, V, C, Wg, b):
    f = np.float32
    U = np.asarray(U, dtype=f)
    V = np.asarray(V, dtype=f)
    C = np.asarray(C, dtype=f)
    Wg = np.asarray(Wg, dtype=f)
    b = np.asarray(b, dtype=f)
    # Vl[i, d, e*R+r] = V[i, e, d, r]
    Vl = np.ascontiguousarray(V.transpose(0, 2, 1, 3).reshape(N_CROSS, D, ER))
    # Ul[i, e*R+r, d] = U[i, e, d, r]
    Ul = np.ascontiguousarray(U.transpose(0, 1, 3, 2).reshape(N_CROSS, ER, D))
    # DoubleRow C: out-block mc pairs rhs h1 blocks (qb, qb+1); the plane
    # matching block mc carries the block-diag expert pair, the other is 0.
    Cb2 = np.zeros((N_CROSS, MC, 2, P, P), dtype=f)
    for i in range(N_CROSS):
        for m in range(MC):
            blk = np.zeros((P, P), dtype=f)
            blk[:R, :R] = C[i, 2 * m]
            blk[R:, R:] = C[i, 2 * m + 1]
            Cb2[i, m, m % 2] = blk
    WgT = np.zeros((D, 2 * E), dtype=f)
    WgT[:, :E] = Wg.T
    # bTc[i, p, kc] = 1 + cumsum_i b  (stt per-partition scalar)
    bc = 1.0 + np.cumsum(b, axis=0)
    bTc = np.ascontiguousarray(bc.reshape(N_CROSS, KC, P).transpose(0, 2, 1))
    sel = np.zeros((E, MC + 1, P), dtype=f)
    for m in range(MC):
        for j in range(P):
            sel[2 * m + j // R, m, j] = 1.0
    sel[:, MC, :] = 1.0
    return dict(
        Vl=Vl.astype(NPF8),
        Ul=Ul.astype(NPF8),
        Cb=Cb2.astype(NPF8),
        WgT=WgT.astype(NPF8),
        bTc=bTc,
        sel=sel.astype(BF16),
        id128=np.eye(P, dtype=f).astype(BF16),
    )


def kernel(x, U, V, C, Wg, b, _trace=False):
    nc = _get_nc()
    w = _prep_weights(U, V, C, Wg, b)
    xs = np.asarray(x, dtype=np.float32).reshape(NCORES, BC, D)
    in_maps = []
    for ci in range(NCORES):
        xt = np.ascontiguousarray(xs[ci].T)
        m = {"xT": xt.astype(BF16), "xT8": xt.astype(NPF8)}
        m.update(w)
        in_maps.append(m)
    res = run_bass_kernel_spmd(nc, in_maps, list(range(NCORES)), trace=_trace)
    kernel.last_result = res
    out = np.concatenate(
        [np.asarray(res.results[ci]["outT"]).astype(np.float32).T
         for ci in range(NCORES)],
        axis=0,
    )
    return np.ascontiguousarray(out, dtype=np.float32)


# revision 27
# speedup vs baseline: 1.9002x; 1.0995x over previous
"""CrossNetMix (DCN-V2 mixture-of-low-rank-experts) Trainium2 kernel.

Data-parallel over batch across 8 cores (2048 rows each); feature-major
([d, b]) on chip so every matmul contraction lands on SBUF partitions.

Matmul stages (gating, V, C, U) run in fp8-e4m3 DoubleRow mode: each
matmul contracts two adjacent 128-row k-subtiles per pass (2 rows/cycle),
roughly halving tensor-engine time vs bf16. PSUM accumulates in fp32.
The softmax/gate-broadcast helper matmuls stay bf16.

Residual reformulation: with S_i = sum_{j<i} (uv_j + b_j),
  xi_i = x0 ⊙ (S_i + 1) .
Each layer's U matmuls accumulate uv into PSUM; for layers > 0 an identity
matmul adds the previous S (bf16, SBUF) into the same accumulation. Then
one scalar_tensor_tensor per d-chunk emits xi = (S + (1 + B_i)) ⊙ x0
directly in fp8 for the next layer's matmuls (bf16 on the last layer for
the output DMA), with the bias cumsum B_i folded into the per-partition
scalar. An ACT copy spills S back to SBUF for the next layer. This keeps
the whole combine at ~2 elementwise ops per d-chunk with no bf16 residual
round-trip on x0.

Chunks are processed in interleaved pairs (A, B): per layer the stream is
A.gating, A.V, A.C, A.U+combine then B.*, so B's matmuls cover A's
combine/softmax tails. Input DMAs are split across both HWDGE queues
(x via Sync/q1, V weights via Scalar/q10) in need-order.
"""

import os
import sys

import ml_dtypes
import numpy as np

if "/opt/trn_rl_repo" not in sys.path:
    sys.path.insert(0, "/opt/trn_rl_repo")

import concourse.bass as bass
import concourse.bacc as bacc
import concourse.mybir as mybir
from concourse.tile import TileContext
from concourse.bass_utils import run_bass_kernel_spmd

AF = mybir.ActivationFunctionType
OP = mybir.AluOpType
DR = mybir.MatmulPerfMode.DoubleRow
F32 = mybir.dt.float32
WDT = mybir.dt.bfloat16
F8 = mybir.dt.float8e4
BF16 = ml_dtypes.bfloat16
NPF8 = ml_dtypes.float8_e4m3

N_CROSS = 3
E = 8            # experts
D = 1024         # feature dim
R = 64           # low rank
B = 16384        # full batch
NCORES = 8
BC = B // NCORES  # rows per core
CHUNK = 512       # batch tile (matmul free dim)
NCHUNK = BC // CHUNK
P = 128
KC = D // P       # d-chunks
ER = E * R        # 512
MC = ER // P      # (e,r)-chunks


def _build():
    nc = bacc.Bacc(None)
    xT = nc.declare_dram_parameter("xT", [D, BC], WDT, isOutput=False)
    xT8 = nc.declare_dram_parameter("xT8", [D, BC], F8, isOutput=False)
    Vl = nc.declare_dram_parameter("Vl", [N_CROSS, D, ER], F8, isOutput=False)
    Cb = nc.declare_dram_parameter("Cb", [N_CROSS, MC, 2, P, P], F8, isOutput=False)
    Ul = nc.declare_dram_parameter("Ul", [N_CROSS, ER, D], F8, isOutput=False)
    # gating weights padded to 16 cols: DoubleRow lhsT outer step must be
    # a multiple of 16 (s3_lw dual-fp8 restriction)
    WgT = nc.declare_dram_parameter("WgT", [D, 2 * E], F8, isOutput=False)
    # bTc[i, p, kc] = 1 + sum_{j<=i} b[j, kc*P+p]  (per-partition stt scalar)
    bTc = nc.declare_dram_parameter("bTc", [N_CROSS, P, KC], F32, isOutput=False)
    sel = nc.declare_dram_parameter("sel", [E, MC + 1, P], WDT, isOutput=False)
    id128 = nc.declare_dram_parameter("id128", [P, P], WDT, isOutput=False)
    outT = nc.declare_dram_parameter("outT", [D, BC], WDT, isOutput=True)

    with TileContext(nc) as tc:
        with (
            tc.sbuf_pool(name="wpool", bufs=1) as wpool,
            tc.sbuf_pool(name="xpool", bufs=4) as xpool,
            tc.sbuf_pool(name="x8pool", bufs=3) as x8pool,
            tc.sbuf_pool(name="xipool", bufs=4) as xipool,
            tc.sbuf_pool(name="spool2", bufs=4) as spool2,
            tc.sbuf_pool(name="xopool", bufs=2) as xopool,
            tc.sbuf_pool(name="h1pool", bufs=2) as h1pool,
            tc.sbuf_pool(name="h2pool", bufs=2) as h2pool,
            tc.sbuf_pool(name="ypool", bufs=2) as ypool,
            tc.sbuf_pool(name="spool", bufs=2) as spool,
            tc.psum_pool(name="psmm", bufs=3) as psmm,
            tc.psum_pool(name="psu", bufs=2) as psu,
            tc.psum_pool(name="pswb", bufs=2) as pswb,
            tc.psum_pool(name="psg", bufs=1) as psg,
        ):
            xTr = xT.rearrange("(kc p) b -> p kc b", p=P)
            xT8r = xT8.rearrange("(kc p) b -> p kc b", p=P)
            outr = outT.rearrange("(kc p) b -> p kc b", p=P)
            Vlr = Vl.rearrange("i (kc p) m -> p i kc m", p=P)
            Ulr = Ul.rearrange("i (mc p) d -> p i mc d", p=P)
            Cbr = Cb.rearrange("i m j p s -> p i m j s")

            def alloc_x0(c):
                t = xpool.tile([P, KC, CHUNK], WDT, tag="x0", name=f"x0_{c}")
                t8 = x8pool.tile([P, KC, CHUNK], F8, tag="x08", name=f"x08_{c}")
                return t, t8

            def load_x8(tt, c):
                cbs = slice(c * CHUNK, (c + 1) * CHUNK)
                nc.sync.dma_start(tt[1], xT8r[:, :, cbs])

            def load_xbf(tt, c):
                cbs = slice(c * CHUNK, (c + 1) * CHUNK)
                nc.sync.dma_start(tt[0], xTr[:, :, cbs])

            def load_x0(c):
                tt = alloc_x0(c)
                load_x8(tt, c)
                load_xbf(tt, c)
                return tt

            wg_sb = wpool.tile([P, KC, 2 * E], F8)
            nc.scalar.dma_start(wg_sb, WgT.rearrange("(kc p) e -> p kc e", p=P))

            v_sb = wpool.tile([P, N_CROSS, KC, ER], F8)
            u_sb = wpool.tile([P, N_CROSS, MC, D], F8)
            c_sb = wpool.tile([P, N_CROSS, MC, 2, P], F8)
            b_sb = wpool.tile([P, N_CROSS, KC], F32)
            id_sb = wpool.tile([P, P], WDT)

            nc.scalar.dma_start(v_sb[:, 0, 0:KC // 2], Vlr[:, 0, 0:KC // 2])
            nc.scalar.dma_start(v_sb[:, 0, KC // 2:], Vlr[:, 0, KC // 2:])
            nc.scalar.dma_start(c_sb, Cbr)

            # q1 in need-order: x0c0 fp8 (gating), U0, x0c1 fp8 (B gating),
            # x0 bf16 copies (first needed at the combine), U1
            x0_tiles = {0: alloc_x0(0), 1: alloc_x0(1)}
            load_x8(x0_tiles[0], 0)
            nc.sync.dma_start(u_sb[:, 0], Ulr[:, 0])
            nc.sync.dma_start(id_sb, id128[:])
            load_x8(x0_tiles[1], 1)
            load_xbf(x0_tiles[0], 0)
            sel_sb = wpool.tile([E, MC + 1, P], WDT)
            nc.scalar.dma_start(sel_sb, sel[:])
            nc.scalar.dma_start(v_sb[:, 1], Vlr[:, 1])
            load_xbf(x0_tiles[1], 1)
            nc.sync.dma_start(u_sb[:, 1], Ulr[:, 1])
            nc.scalar.dma_start(b_sb, bTc.rearrange("i p kc -> p i kc"))
            nc.scalar.dma_start(v_sb[:, 2], Vlr[:, 2])
            nc.scalar.dma_start(u_sb[:, 2], Ulr[:, 2])

            s_tiles = {}
            xi8_tiles = {}
            for pair in range(NCHUNK // 2):
                for i in range(N_CROSS):
                    for half in range(2):
                        c = 2 * pair + half
                        if i == 1 and c + 2 < NCHUNK:
                            x0_tiles[c + 2] = load_x0(c + 2)
                        bs = slice(c * CHUNK, (c + 1) * CHUNK)
                        x0, x08 = x0_tiles[c]
                        src8 = x08 if i == 0 else xi8_tiles.pop(c)
                        s_prev = None if i == 0 else s_tiles.pop(c)
                        last = i == N_CROSS - 1
                        if last:
                            xi = xopool.tile([P, KC, CHUNK], WDT, tag="xiout")
                            x0_tiles.pop(c)
                        else:
                            xi = xipool.tile([P, KC, CHUNK], F8, tag="xi8")
                            xi8_tiles[c] = xi
                            s_new = spool2.tile([P, KC, CHUNK], WDT, tag="s")
                            s_tiles[c] = s_new
                        # ---- gating (fp8 DoubleRow, 16-col padded) ----
                        gps = psg.tile([2 * E, CHUNK], F32, tag="g")
                        for q in range(KC // 2):
                            nc.tensor.matmul(
                                gps,
                                wg_sb[:, 2 * q : 2 * q + 2, :],
                                src8[:, 2 * q : 2 * q + 2, :],
                                start=(q == 0),
                                stop=(q == KC // 2 - 1),
                                perf_mode=DR,
                            )
                        expg = spool.tile([E, CHUNK], WDT, tag="expg")
                        nc.scalar.activation(expg, gps[0:E, :], AF.Exp)
                        # ---- V stage (fp8 DoubleRow) ----
                        h1 = h1pool.tile([P, MC, CHUNK], F8, tag="h1")
                        for mc in range(MC):
                            vps = psmm.tile([P, CHUNK], F32, tag="mm")
                            for q in range(KC // 2):
                                nc.tensor.matmul(
                                    vps,
                                    v_sb[:, i, 2 * q : 2 * q + 2,
                                         mc * P : (mc + 1) * P],
                                    src8[:, 2 * q : 2 * q + 2, :],
                                    start=(q == 0),
                                    stop=(q == KC // 2 - 1),
                                    perf_mode=DR,
                                )
                            nc.scalar.activation(h1[:, mc, :], vps, AF.Tanh)
                        # ---- softmax tail ----
                        sums = psg.tile([1, CHUNK], F32, tag="g")
                        nc.tensor.matmul(
                            sums, sel_sb[:, MC, 0:1], expg, start=True, stop=True
                        )
                        rfast = spool.tile([1, CHUNK], F32, tag="rfast")
                        nc.vector.reciprocal_approx_fast(rfast, sums)
                        rrow = spool.tile([1, CHUNK], WDT, tag="rrow")
                        nc.vector.tensor_copy(rrow, rfast)
                        wps = psg.tile([E, CHUNK], F32, tag="g")
                        nc.tensor.matmul(
                            wps, sel_sb[0:1, MC, 0:E], rrow, start=True, stop=True
                        )
                        wsb = spool.tile([E, CHUNK], WDT, tag="wsb")
                        nc.vector.tensor_tensor(wsb, expg, wps, OP.mult)
                        # ---- C stage (fp8 DoubleRow, zero-padded pair) ----
                        ys = ypool.tile([P, MC, CHUNK], F8, tag="y")
                        for mc in range(MC):
                            cps = psmm.tile([P, CHUNK], F32, tag="mm")
                            qb = (mc // 2) * 2
                            nc.tensor.matmul(
                                cps,
                                c_sb[:, i, mc, :, :],
                                h1[:, qb : qb + 2, :],
                                start=True,
                                stop=True,
                                perf_mode=DR,
                            )
                            wbp = pswb.tile([P, CHUNK], F32, tag="wb")
                            nc.tensor.matmul(
                                wbp, sel_sb[:, mc, :], wsb, start=True, stop=True
                            )
                            h2 = h2pool.tile([P, CHUNK], F32, tag="h2")
                            nc.scalar.activation(h2, cps, AF.Tanh)
                            nc.vector.tensor_tensor(ys[:, mc, :], h2, wbp, OP.mult)
                        # ---- U stage + S accumulate + combine ----
                        for dc in range(KC):
                            ups = psu.tile([P, CHUNK], F32, tag="u")
                            for q in range(MC // 2):
                                nc.tensor.matmul(
                                    ups,
                                    u_sb[:, i, 2 * q : 2 * q + 2,
                                         dc * P : (dc + 1) * P],
                                    ys[:, 2 * q : 2 * q + 2, :],
                                    start=(q == 0),
                                    stop=(s_prev is None and q == MC // 2 - 1),
                                    perf_mode=DR,
                                )
                            if s_prev is not None:
                                nc.tensor.matmul(
                                    ups, id_sb, s_prev[:, dc, :],
                                    start=False, stop=True,
                                )
                            # xi = (S + (1 + B_i)) * x0
                            nc.vector.scalar_tensor_tensor(
                                xi[:, dc, :],
                                ups,
                                b_sb[:, i, dc : dc + 1],
                                x0[:, dc, :],
                                OP.add,
                                OP.mult,
                            )
                            if not last:
                                nc.scalar.activation(
                                    s_new[:, dc, :], ups, AF.Copy
                                )
                            else:
                                eng = nc.sync if dc % 2 == 0 else nc.scalar
                                eng.dma_start(outr[:, dc, bs], xi[:, dc, :])
    nc.compile()
    return nc


_CTX = {}


def _get_nc():
    if "nc" not in _CTX:
        _CTX["nc"] = _build()
    return _CTX["nc"]


def _prep_weights(U, V, C, Wg, b):
    f = np.float32
    U = np.asarray(U, dtype=f)
    V = np.asarray(V, dtype=f)
    C = np.asarray(C, dtype=f)
    Wg = np.asarray(Wg, dtype=f)
    b = np.asarray(b, dtype=f)
    # Vl[i, d, e*R+r] = V[i, e, d, r]
    Vl = np.ascontiguousarray(V.transpose(0, 2, 1, 3).reshape(N_CROSS, D, ER))
    # Ul[i, e*R+r, d] = U[i, e, d, r]
    Ul = np.ascontiguousarray(U.transpose(0, 1, 3, 2).reshape(N_CROSS, ER, D))
    # DoubleRow C: out-block mc pairs rhs h1 blocks (qb, qb+1); the plane
    # matching block mc carries the block-diag expert pair, the other is 0.
    Cb2 = np.zeros((N_CROSS, MC, 2, P, P), dtype=f)
    for i in range(N_CROSS):
        for m in range(MC):
            blk = np.zeros((P, P), dtype=f)
            blk[:R, :R] = C[i, 2 * m]
            blk[R:, R:] = C[i, 2 * m + 1]
            Cb2[i, m, m % 2] = blk
    WgT = np.zeros((D, 2 * E), dtype=f)
    WgT[:, :E] = Wg.T
    # bTc[i, p, kc] = 1 + cumsum_i b  (stt per-partition scalar)
    bc = 1.0 + np.cumsum(b, axis=0)
    bTc = np.ascontiguousarray(bc.reshape(N_CROSS, KC, P).transpose(0, 2, 1))
    sel = np.zeros((E, MC + 1, P), dtype=f)
    for m in range(MC):
        for j in range(P):
            sel[2 * m + j // R, m, j] = 1.0
    sel[:, MC, :] = 1.0
    return dict(
        Vl=Vl.astype(NPF8),
        Ul=Ul.astype(NPF8),
        Cb=Cb2.astype(NPF8),
        WgT=WgT.astype(NPF8),
        bTc=bTc,
        sel=sel.astype(BF16),
        id128=np.eye(P, dtype=f).astype(BF16),
    )


def kernel(x, U, V, C, Wg, b, _trace=False):
    nc = _get_nc()
    w = _prep_weights(U, V, C, Wg, b)
    xs = np.asarray(x, dtype=np.float32).reshape(NCORES, BC, D)
    in_maps = []
    for ci in range(NCORES):
        xt = np.ascontiguousarray(xs[ci].T)
        m = {"xT": xt.astype(BF16), "xT8": xt.astype(NPF8)}
        m.update(w)
        in_maps.append(m)
    res = run_bass_kernel_spmd(nc, in_maps, list(range(NCORES)), trace=_trace)
    kernel.last_result = res
    out = np.concatenate(
        [np.asarray(res.results[ci]["outT"]).astype(np.float32).T
         for ci in range(NCORES)],
        axis=0,
    )
    return np.ascontiguousarray(out, dtype=np.float32)
